# revision 1
# baseline (speedup 1.0000x reference)
"""Trainium2 Bass kernel for nn_CWAUCHLoss (pairwise AUC hinge + class-weighted CE).

Math: with s = sigmoid(output[:, 0]), lab = labels[:, 0], the O(B^2) pairwise
penalty collapses algebraically (LAMB == 2):

  sum_{i in N, j in P} (1 - (s_j - s_i))^2
    = n_pos * sum_N s^2 + 2 * (sum_N s)(sum_P (1-s)) + n_neg * sum_P (1-s)^2

so the whole loss reduces to 7 masked sums over B elements:
  r0 = sum lab          r1 = sum s       r2 = sum s^2
  r3 = sum lab*s        r4 = sum lab*s^2
  q1 = sum ln(1+e^-s)   q2 = sum lab*ln(1+e^-s)
using ln(t) = -ln(1+e^-s) and ln(1-t) = ln(t) - s for t = sigmoid(s).

On-chip (one NeuronCore, batch spread over 128 partitions x 64 lanes):
  - sigmoid is computed as 1/(1+exp(-x)) so ALL transcendentals (Exp, Ln)
    live in one ACT table set (a Sigmoid->Ln sequence would pay a ~2.7us
    mid-kernel table switch); a dummy Exp prewarms the table during the
    input DMA, and a post-compile pass retargets the auto-inserted table
    load to the combined natural_log_exp_and_others set.
  - per-partition sums land in a [128, 8] stats tile via activation/STT
    accumulators; a matmul against the tile's own 1/128 constant column
    reduces across partitions; a second matmul against a constant 8x12
    matrix forms every linear combination; a 5-product bilinear form +
    grouped reduce yields [numerator, fpcls].
  - the penalty denominator 2*r0*(B-r0) depends only on the label count,
    which lands ~1.3us before the ln chain finishes, so 1/den is computed
    on a DVE side chain (tiny r0-only matmul -> affine -> reciprocal)
    entirely inside that idle window, off the critical tail.
  - raw Bass (nc.Block) with per-engine counter semaphores: TRN2 engines
    are deep-pipelined with no scoreboard, so every RAW dependency (same-
    engine included) is sequenced through semaphores; skipping Tile's
    entry/exit barriers saves ~0.4us on a ~8.4us kernel.
"""

import numpy as np

B = 8192
P = 128
N = B // P  # 64 elements per partition

_nc_cache = None


def _wmat() -> np.ndarray:
    # Rows index the partition-reduced stats rc = [r0,r1,r2,r3,r4,1,q1,q2]/128.
    # Cols 0-5 build vector A, cols 6-11 build vector B; elementwise A*B then
    # group-sum by 3 gives [penalty numerator, fpcls] (the denominator is
    # handled by the r0-only side chain).
    W = np.zeros((8, 12), dtype=np.float64)
    Bf = float(B)
    W[0, 0] = 1.0                                   # A0 = r0
    W[1, 1] = 2.0
    W[3, 1] = -2.0                                  # A1 = 2(r1-r3)
    W[5, 2] = Bf
    W[0, 2] = -1.0                                  # A2 = B-r0
    W[5, 3] = 1.0 / Bf                              # A3 = 1/B
    W[0, 4] = 1.0 / (Bf * Bf)                       # A4 = r0/B^2
    W[2, 6] = 1.0
    W[4, 6] = -1.0                                  # B0 = r2-r4
    W[0, 7] = 1.0
    W[3, 7] = -1.0                                  # B1 = r0-r3
    W[0, 8] = 1.0
    W[3, 8] = -2.0
    W[4, 8] = 1.0                                   # B2 = r0-2r3+r4
    W[7, 9] = 1.0                                   # B3 = q2
    W[6, 10] = 1.0
    W[1, 10] = 1.0
    W[7, 10] = -2.0
    W[3, 10] = -1.0                                 # B4 = q1+r1-2q2-r3
    # rc carries true_sums/128 (the reduce matmul weights by the 1/128 const
    # column), so scale every coefficient by 128 to compensate.
    return np.ascontiguousarray(W * P, dtype=np.float32)


def build_nc():
    from contextlib import ExitStack

    import concourse.bacc as bacc
    import concourse.mybir as mybir

    f32 = mybir.dt.float32
    AF = mybir.ActivationFunctionType
    ALU = mybir.AluOpType
    AX = mybir.AxisListType

    nc = bacc.Bacc(None, target_bir_lowering=False, debug=False)
    x_d = nc.dram_tensor("output", [B, 2], f32, kind="ExternalInput")
    l_d = nc.dram_tensor("labels", [B, 1], f32, kind="ExternalInput")
    w_d = nc.dram_tensor("wmat", [8, 12], f32, kind="ExternalInput")
    o_d = nc.dram_tensor("out", [1, 2], f32, kind="ExternalOutput")

    with ExitStack() as ctx:
        e = ctx.enter_context
        xt = e(nc.sbuf_tensor([P, N, 2], f32))
        lt = e(nc.sbuf_tensor([P, N], f32))
        wt = e(nc.sbuf_tensor([8, 12], f32))
        e1 = e(nc.sbuf_tensor([P, N], f32))
        p1 = e(nc.sbuf_tensor([P, N], f32))
        s = e(nc.sbuf_tensor([P, N], f32))
        ls = e(nc.sbuf_tensor([P, N], f32))
        e2 = e(nc.sbuf_tensor([P, N], f32))
        lnw = e(nc.sbuf_tensor([P, N], f32))
        scr2 = e(nc.sbuf_tensor([P, N], f32))
        scr4 = e(nc.sbuf_tensor([P, N], f32))
        scrq = e(nc.sbuf_tensor([P, N], f32))
        ST = e(nc.sbuf_tensor([P, 8], f32))
        warm = e(nc.sbuf_tensor([1, 2], f32))
        bias01 = e(nc.sbuf_tensor([P, 2], f32))
        rcs = e(nc.sbuf_tensor([8, 1], f32))
        LCs = e(nc.sbuf_tensor([1, 12], f32))
        PPt = e(nc.sbuf_tensor([1, 6], f32))
        Ft = e(nc.sbuf_tensor([1, 4], f32))
        r0s = e(nc.sbuf_tensor([1, 4], f32))  # [r0/128, t1, den, invden]
        G = e(nc.sbuf_tensor([1, 2], f32))
        psA = e(nc.psum_tensor([8, 1], f32))
        psB = e(nc.psum_tensor([1, 12], f32))
        psR = e(nc.psum_tensor([1, 1], f32))
        d_x = e(nc.semaphore("d_x"))
        d_l = e(nc.semaphore("d_l"))
        d_w = e(nc.semaphore("d_w"))
        d_o = e(nc.semaphore("d_o"))
        ACTc = e(nc.semaphore("ACTc"))
        DVEc = e(nc.semaphore("DVEc"))
        PEc = e(nc.semaphore("PEc"))
        block = e(nc.Block())

        @block.sync
        def _(sync):
            # x first: it gates the whole compute chain (HWDGE descriptor
            # generation is a shared serial unit, ~625ns per dma_start).
            sync.dma_start(
                xt[:], x_d.ap().rearrange("(p n) c -> p n c", p=P)
            ).then_inc(d_x, 16)
            sync.dma_start(
                lt[:], l_d.ap().rearrange("(p n) c -> p (n c)", p=P)
            ).then_inc(d_l, 16)
            sync.wait_ge(DVEc, 22)  # G written
            sync.dma_start(o_d.ap(), G[:]).then_inc(d_o, 16)
            sync.wait_ge(d_o, 16)

        @block.gpsimd
        def _(gpsimd):
            # wmat is needed late (second matmul); SWDGE generation on the
            # otherwise-idle Pool engine runs parallel to the HWDGE unit.
            gpsimd.dma_start(wt[:], w_d.ap()).then_inc(d_w, 16)

        @block.scalar
        def _(scalar):
            scalar.wait_ge(DVEc, 3)  # bias01 + warm tile memsets
            # prewarm: pulls the exp/ln table set during the input DMA
            scalar.activation(
                warm[:], warm[:], AF.Exp, bias=bias01[0:1, 0:1]
            ).then_inc(ACTc, 1)  # 1
            scalar.wait_ge(d_x, 16)
            scalar.activation(
                e1[:], xt[:, :, 0], AF.Exp, scale=-1.0, bias=bias01[:, 0:1]
            ).then_inc(ACTc, 1)  # 2
            scalar.activation(
                e2[:], s[:], AF.Exp, scale=-1.0, bias=bias01[:, 0:1]
            ).then_inc(ACTc, 1)._wait_ge(DVEc, 7)  # 3
            # ln(1+e2): the +1 rides the Ln op's bias input; accum -> q1
            scalar.activation(
                lnw[:], e2[:], AF.Ln, bias=bias01[:, 1:2],
                accum_out=ST[:, 6:7],
            ).then_inc(ACTc, 1)._wait_ge(ACTc, 3)  # 4

        @block.vector
        def _(vector):
            # dep-free preamble memsets (compute path => inc-by-1 legal;
            # gpsimd memsets with inc-1 sems crash the device)
            vector.memset(bias01[:, 0:1], 0.0).then_inc(DVEc, 1)   # 1
            vector.memset(bias01[:, 1:2], 1.0).then_inc(DVEc, 1)   # 2
            vector.memset(warm[:], 1.0).then_inc(DVEc, 1)          # 3
            vector.memset(Ft[:, 2:3], 0.0).then_inc(DVEc, 1)       # 4
            vector.memset(ST[:, 5:6], 1.0 / P).then_inc(DVEc, 1)   # 5
            # s = sigmoid(x0) = 1/(1+e1); reciprocal on DVE is IEEE-exact
            vector.tensor_scalar_add(
                p1[:], e1[:], 1.0
            ).then_inc(DVEc, 1)._wait_ge(ACTc, 2)  # 6
            vector.reciprocal(s[:], p1[:]).then_inc(DVEc, 1)._wait_ge(DVEc, 6)  # 7
            # per-partition stats (fill DVE idle time under the ACT chain);
            # r0 first: it feeds the invden side chain below
            vector.wait_ge(d_l, 16)
            vector.tensor_reduce(
                ST[:, 0:1], lt[:], axis=AX.X, op=ALU.add
            ).then_inc(DVEc, 1)  # 8
            vector.tensor_reduce(
                ST[:, 1:2], s[:], axis=AX.X, op=ALU.add
            ).then_inc(DVEc, 1)._wait_ge(DVEc, 7)  # 9
            vector.scalar_tensor_tensor(
                out=ls[:], in0=lt[:], scalar=1.0, in1=s[:],
                op0=ALU.mult, op1=ALU.mult, accum_out=ST[:, 3:4],
            ).then_inc(DVEc, 1)  # 10
            vector.scalar_tensor_tensor(
                out=scr2[:], in0=s[:], scalar=1.0, in1=s[:],
                op0=ALU.mult, op1=ALU.mult, accum_out=ST[:, 2:3],
            ).then_inc(DVEc, 1)  # 11
            vector.scalar_tensor_tensor(
                out=scr4[:], in0=ls[:], scalar=1.0, in1=ls[:],
                op0=ALU.mult, op1=ALU.mult, accum_out=ST[:, 4:5],
            ).then_inc(DVEc, 1)._wait_ge(DVEc, 10)  # 12
            # invden side chain: den = 2*r0*(B-r0) depends only on r0, so
            # 1/den is ready long before q2 and leaves the critical tail.
            # rc0 = r0/128, so den = rc0*(2*128*B - 2*128^2*rc0); all
            # coefficients are powers of two (exact in f32).
            vector.tensor_copy(r0s[0:1, 0:1], psR[:]).then_inc(DVEc, 1)._wait_ge(PEc, 1)  # 13
            vector.tensor_scalar(
                out=r0s[0:1, 1:2], in0=r0s[0:1, 0:1],
                scalar1=-2.0 * 128.0 * 128.0, scalar2=2.0 * 128.0 * 8192.0,
                op0=ALU.mult, op1=ALU.add,
            ).then_inc(DVEc, 1)._wait_ge(DVEc, 13)  # 14
            vector.tensor_tensor(
                r0s[0:1, 2:3], r0s[0:1, 0:1], r0s[0:1, 1:2], op=ALU.mult
            ).then_inc(DVEc, 1)._wait_ge(DVEc, 14)  # 15
            vector.reciprocal(
                r0s[0:1, 3:4], r0s[0:1, 2:3]
            ).then_inc(DVEc, 1)._wait_ge(DVEc, 15)  # 16
            # last stat: q2 (gated by the ln chain)
            vector.scalar_tensor_tensor(
                out=scrq[:], in0=lt[:], scalar=1.0, in1=lnw[:],
                op0=ALU.mult, op1=ALU.mult, accum_out=ST[:, 7:8],
            ).then_inc(DVEc, 1)._wait_ge(ACTc, 4)  # 17
            # tail: PSUM staging copies, bilinear products, grouped sums,
            # then G = [num*invden + fpcls, num*invden] = [cls, penalty]
            vector.tensor_copy(rcs[:], psA[:]).then_inc(DVEc, 1)._wait_ge(PEc, 2)  # 18
            vector.tensor_copy(LCs[:], psB[:]).then_inc(DVEc, 1)._wait_ge(PEc, 3)  # 19
            vector.tensor_tensor(
                PPt[:], LCs[0:1, 0:6], LCs[0:1, 6:12], op=ALU.mult
            ).then_inc(DVEc, 1)._wait_ge(DVEc, 19)  # 20
            vector.tensor_reduce(
                Ft[:, 0:2],
                PPt[:].rearrange("p (g k) -> p g k", k=3),
                axis=AX.X,
                op=ALU.add,
            ).then_inc(DVEc, 1)._wait_ge(DVEc, 20)  # 21
            vector.scalar_tensor_tensor(
                out=G[:],
                in0=Ft[:, 0:1].broadcast_to([1, 2]),
                scalar=r0s[0:1, 3:4],
                in1=Ft[:, 1:3],
                op0=ALU.mult,
                op1=ALU.add,
            ).then_inc(DVEc, 1)._wait_ge(DVEc, 21)  # 22

        @block.tensor
        def _(tensor):
            # r0-only reduce for the invden side chain (r0 lands early)
            tensor.matmul(
                psR[:], ST[:, 0:1], ST[:, 5:6]
            ).then_inc(PEc, 1)._wait_ge(DVEc, 8)
            tensor.wait_ge(ACTc, 4)   # lnw accum (q1)
            # cross-partition reduce: rc = ST^T @ (1/128 column)
            tensor.matmul(
                psA[:], ST[:, 0:8], ST[:, 5:6]
            ).then_inc(PEc, 1)._wait_ge(DVEc, 17)
            tensor.wait_ge(d_w, 16)   # wt
            # all linear combos: LC = rc^T @ W
            tensor.matmul(
                psB[:], rcs[:], wt[:]
            ).then_inc(PEc, 1)._wait_ge(DVEc, 18)

    nc.compile()

    # Table-load surgery: the greedy chooser assigns set 0 (exp_and_others)
    # to the Exp ops and then pays a second mid-chain ~1.3us load of set 5
    # (natural_log) before Ln.  Set 6 (natural_log_exp_and_others) contains
    # BOTH, so retarget the first load and drop the rest (they carry no
    # semaphore waits/updates).
    _COMBINED_EXP_LN_SET = 6
    for blk in nc.main_func.blocks:
        loads = [
            i for i in blk.instructions
            if isinstance(i, mybir.InstLoadActFuncSet)
        ]
        if not loads:
            continue
        assert all(not i.has_wait() and not i.has_update() for i in loads)
        loads[0].act_func_set_id = _COMBINED_EXP_LN_SET
        drop = {id(i) for i in loads[1:]}
        kept = [i for i in blk.instructions if id(i) not in drop]
        del blk.instructions[:]
        blk.instructions.extend(kept)

    # Drop Bass.__init__'s unconditional const-AP memsets (f32 0/1, bf16 1,
    # u8 127): nothing in this kernel reads them (biases come from bias01).
    import json as _json

    for blk in nc.main_func.blocks:
        kept = []
        for i in blk.instructions:
            if isinstance(i, mybir.InstMemset) and not i.has_wait() and not i.has_update():
                j = _json.loads(mybir.instruction_to_pretty_json_string(i))
                memref = j.get("outs", [{}])[0].get("memref", "")
                if isinstance(memref, str) and memref.startswith("const-"):
                    continue
            kept.append(i)
        if len(kept) != len(blk.instructions):
            del blk.instructions[:]
            blk.instructions.extend(kept)
    return nc


def _in_map(output: np.ndarray, labels: np.ndarray) -> dict:
    return {
        "output": np.ascontiguousarray(output, dtype=np.float32),
        "labels": np.ascontiguousarray(labels, dtype=np.float32),
        "wmat": _wmat(),
    }


def kernel(output: np.ndarray, labels: np.ndarray) -> np.ndarray:
    global _nc_cache
    from concourse.bass_utils import run_bass_kernel_spmd

    if _nc_cache is None:
        _nc_cache = build_nc()
    res = run_bass_kernel_spmd(_nc_cache, [_in_map(output, labels)], core_ids=[0])
    g = res.results[0]["out"]
    return np.asarray(g, dtype=np.float32).reshape(2).copy()



# revision 10
# speedup vs baseline: 1.0754x; 1.0754x over previous
"""Trainium2 Bass kernel for nn_CWAUCHLoss (pairwise AUC hinge + class-weighted CE).

Math: with s = sigmoid(output[:, 0]), lab = labels[:, 0], LAMB == 2, the
O(B^2) pairwise penalty collapses to 3 pairwise products of masked sums,
and the CE part is linear in the two log-sums given the label count:

  num  = r0*C2 + 2*C1*C3 + (B-r0)*C4      pen = num / (2*r0*(B-r0))
  fpcls = alpha*q1 + beta*q2 + alpha*C1   cls = fpcls + pen
  where C1 = sum s*(1-lab)       C2 = sum s^2*(1-lab)
        C3 = sum lab*(1-s)       C4 = sum lab*(1-s)^2      r0 = sum lab
        q1 = sum ln(1+e^-s)      q2 = sum lab*ln(1+e^-s)
        alpha = r0/B^2           beta = (B-2*r0)/B^2

Device schedule (one NeuronCore, batch as 128 partitions x 64 lanes):
  - per-partition combo-columns accumulate in ST2[128, 9] laid out as
    [r0, C2, -2C1, -C3, 64-r0_p, C4 | -2C1, q1, q2] so that a single
    stride-2-paired STT dot gives num, and a single 3-term STT dot against
    [-a/2, a, b] gives fpcls (sign folds make every op expressible as one
    scalar_tensor_tensor).
  - cross-partition reduction is two tiny matmuls against a ones column
    (cols 0:6 -> psR as soon as C4 lands, cols 6:9 -> psQ after q2); an
    r0-only matmul runs early so 1/den, alpha, beta are ready off-path.
  - the output travels through a PREPARED SWDGE scatter-add descriptor:
    desc-gen (~1us Pool time) happens during the input DMA; after G is
    written, trigger_dma fires the 256B payload immediately - skipping
    the 625ns HWDGE desc-gen + 650ns DGE delay a fresh dma_start pays.
    o_d is zero-initialized by an early HWDGE DMA so add == store.
  - ALL transcendentals (Exp, Ln) use act table set 6
    (natural_log_exp_and_others); post-compile surgery retargets the
    auto-inserted table load (same trick as before).
  - post-compile surgery also strips Bass's entry/exit barrier protocol
    (gather/release EventSemaphores + Drain sync): single-shot kernel,
    engines start immediately and halt after their last instruction.
"""

import numpy as np

B = 8192
P = 128
N = B // P  # 64 elements per partition

_nc_cache = None


def build_nc():
    from contextlib import ExitStack

    import concourse.bacc as bacc
    import concourse.mybir as mybir

    f32 = mybir.dt.float32
    i16 = mybir.dt.int16
    AF = mybir.ActivationFunctionType
    ALU = mybir.AluOpType
    AX = mybir.AxisListType

    nc = bacc.Bacc(None, target_bir_lowering=False, debug=False)
    x_d = nc.dram_tensor("output", [B, 2], f32, kind="ExternalInput")
    l_d = nc.dram_tensor("labels", [B, 1], f32, kind="ExternalInput")
    o_d = nc.dram_tensor("out", [1, 64], f32, kind="ExternalOutput")

    Bf = float(B)

    with ExitStack() as ctx:
        e = ctx.enter_context
        xt = e(nc.sbuf_tensor([P, N, 2], f32))
        lt = e(nc.sbuf_tensor([P, N], f32))
        e1 = e(nc.sbuf_tensor([P, N], f32))
        p1 = e(nc.sbuf_tensor([P, N], f32))
        s = e(nc.sbuf_tensor([P, N], f32))
        e2 = e(nc.sbuf_tensor([P, N], f32))
        lnw = e(nc.sbuf_tensor([P, N], f32))
        ls2 = e(nc.sbuf_tensor([P, N], f32))
        scr = e(nc.sbuf_tensor([P, N], f32))
        b1 = e(nc.sbuf_tensor([P, N], f32))
        d1 = e(nc.sbuf_tensor([P, N], f32))
        d2 = e(nc.sbuf_tensor([P, N], f32))
        d3 = e(nc.sbuf_tensor([P, N], f32))
        ST2 = e(nc.sbuf_tensor([P, 9], f32))
        onescol = e(nc.sbuf_tensor([P, 1], f32))
        zerocol = e(nc.sbuf_tensor([P, 1], f32))
        zsrc = e(nc.sbuf_tensor([1, 64], f32))
        Gt = e(nc.sbuf_tensor([P, 64], f32))
        idxS = e(nc.sbuf_tensor([P, 1], i16))
        qcoef = e(nc.sbuf_tensor([1, 3], f32))
        r0s = e(nc.sbuf_tensor([1, 4], f32))  # [t1, den, invden, spare]
        Ft = e(nc.sbuf_tensor([1, 1], f32))
        QG = e(nc.sbuf_tensor([1, 2], f32))
        rc6 = e(nc.sbuf_tensor([1, 6], f32))
        prodA = e(nc.sbuf_tensor([1, 3, 1], f32))
        prodB = e(nc.sbuf_tensor([1, 3], f32))
        psR0 = e(nc.psum_tensor([1, 1], f32))
        psR = e(nc.psum_tensor([1, 6], f32))
        psQ = e(nc.psum_tensor([1, 3], f32))
        d_x = e(nc.semaphore("d_x"))
        d_l = e(nc.semaphore("d_l"))
        d_z = e(nc.semaphore("d_z"))
        d_o = e(nc.semaphore("d_o"))
        Pp = e(nc.semaphore("Pp"))
        Pd = e(nc.semaphore("Pd"))
        ACTc = e(nc.semaphore("ACTc"))
        DVEc = e(nc.semaphore("DVEc"))
        PEc = e(nc.semaphore("PEc"))
        block = e(nc.Block())

        import os
        out_mode = os.environ.get("OUT_MODE", "trig")

        @block.sync
        def _(sync):
            # x first: it gates the whole compute chain.
            sync.dma_start(
                xt[:], x_d.ap().rearrange("(p n) c -> p n c", p=P)
            ).then_inc(d_x, 16)
            sync.dma_start(
                lt[:], l_d.ap().rearrange("(p n) c -> p (n c)", p=P)
            ).then_inc(d_l, 16)
            if out_mode == "dma":
                sync.wait_ge(DVEc, 26)
                sync.dma_start(o_d.ap(), Gt[0:1, :]).then_inc(d_o, 16)
                sync.wait_ge(d_o, 16)
                return
            # zero-init the output row so the scatter-ADD acts as a store
            sync.wait_ge(DVEc, 1)
            sync.dma_start(o_d.ap(), zsrc[:]).then_inc(d_z, 16)

        @block.gpsimd
        def _(gpsimd):
            if out_mode == "trig":
                # Pre-generate the output descriptor during the input DMA
                # window; trigger_dma later fires it with no desc-gen on the
                # tail.
                gpsimd.wait_ge(DVEc, 6)
                gpsimd.dma_scatter_add(
                    out_ap=o_d.ap(),
                    in_ap=Gt[:].rearrange("p (a b) -> p a b", a=1),
                    idxs_ap=idxS[:, 0:1],
                    num_idxs=1,
                    num_idxs_reg=1,
                    elem_size=64,
                    prepare_only=True,
                    sem=d_o,
                ).then_inc(Pp, 16)
            # duplicate the -2C1 accum column into the num-pair slot (Pool is
            # idle here; keeps the DVE column pipeline one op shorter)
            gpsimd.tensor_scalar(
                out=ST2[:, 2:3], in0=ST2[:, 6:7], scalar1=1.0, scalar2=None,
                op0=ALU.mult,
            ).then_inc(Pd, 16)._wait_ge(DVEc, 14)
            if out_mode == "trig":
                gpsimd.wait_ge(Pp, 16)
                gpsimd.wait_ge(d_z, 16)
                gpsimd.trigger_dma(count=1)._wait_ge(DVEc, 26)
                gpsimd.wait_ge(d_o, 16)
            elif out_mode == "scat":
                gpsimd.wait_ge(d_z, 16)
                gpsimd.wait_ge(DVEc, 26)
                gpsimd.dma_scatter_add(
                    out_ap=o_d.ap(),
                    in_ap=Gt[:].rearrange("p (a b) -> p a b", a=1),
                    idxs_ap=idxS[:, 0:1],
                    num_idxs=1,
                    num_idxs_reg=1,
                    elem_size=64,
                ).then_inc(d_o, 16)
                gpsimd.wait_ge(d_o, 16)

        @block.scalar
        def _(scalar):
            scalar.wait_ge(DVEc, 3)  # zerocol/onescol biases
            scalar.activation(
                e1[:], xt[:, :, 0], AF.Exp, scale=-1.0, bias=zerocol[:, 0:1]
            ).then_inc(ACTc, 1)._wait_ge(d_x, 16)  # 1
            scalar.activation(
                e2[:], s[:], AF.Exp, scale=-1.0, bias=zerocol[:, 0:1]
            ).then_inc(ACTc, 1)._wait_ge(DVEc, 10)  # 2
            scalar.activation(
                lnw[:], e2[:], AF.Ln, bias=onescol[:, 0:1],
                accum_out=ST2[:, 7:8],
            ).then_inc(ACTc, 1)._wait_ge(ACTc, 2)  # 3 (accum -> q1)
            # stage psR to SBUF for the num dot (DVE may read only one PSUM
            # operand per op; ACT is idle here and reads PSUM fast)
            scalar.activation(
                rc6[:], psR[:], AF.Copy, scale=1.0, bias=0.0
            ).then_inc(ACTc, 1)._wait_ge(PEc, 2)  # 4

        @block.vector
        def _(vector):
            # DVEc is a monotone completion counter: a wait >= k orders this
            # op after ALL DVE ops <= k. Waits target op k-2 or earlier
            # wherever possible (free in the pipeline); only true
            # immediately-adjacent RAW pairs pay the sem round-trip.
            vector.memset(zsrc[:], 0.0).then_inc(DVEc, 1)              # 1
            vector.memset(onescol[:], 1.0).then_inc(DVEc, 1)._wait_ge(DVEc, 1)   # 2
            vector.memset(zerocol[:], 0.0).then_inc(DVEc, 1)._wait_ge(DVEc, 2)   # 3
            vector.memset(QG[:], 0.0).then_inc(DVEc, 1)._wait_ge(DVEc, 3)        # 4
            vector.memset(idxS[:], -1).then_inc(DVEc, 1)._wait_ge(DVEc, 4)       # 5
            vector.memset(idxS[0:1, 0:1], 0).then_inc(DVEc, 1)._wait_ge(DVEc, 5) # 6
            vector.memset(Gt[:], 0.0).then_inc(DVEc, 1)._wait_ge(DVEc, 6)        # 7
            # s = sigmoid(x0) = 1/(1+e1)
            vector.wait_ge(DVEc, 7)
            vector.tensor_scalar_add(
                p1[:], e1[:], 1.0
            ).then_inc(DVEc, 1)._wait_ge(ACTc, 1)                      # 8
            vector.wait_ge(DVEc, 8)
            vector.tensor_reduce(
                ST2[:, 0:1], lt[:], axis=AX.X, op=ALU.add
            ).then_inc(DVEc, 1)._wait_ge(d_l, 16)                      # 9 r0
            vector.reciprocal(s[:], p1[:]).then_inc(DVEc, 1)._wait_ge(DVEc, 8)  # 10
            vector.tensor_scalar(
                out=ST2[:, 4:5], in0=ST2[:, 0:1], scalar1=-1.0, scalar2=float(N),
                op0=ALU.mult, op1=ALU.add,
            ).then_inc(DVEc, 1)._wait_ge(DVEc, 9)                      # 11 Bmr col
            vector.scalar_tensor_tensor(
                out=ls2[:], in0=lt[:], scalar=2.0, in1=s[:],
                op0=ALU.mult, op1=ALU.mult,
            ).then_inc(DVEc, 1)._wait_ge(DVEc, 10)                     # 12 ls2=2*lab*s
            vector.tensor_scalar(
                out=qcoef[0:1, 1:2], in0=psR0[:], scalar1=1.0 / (Bf * Bf),
                scalar2=None, op0=ALU.mult,
            ).then_inc(DVEc, 1)._wait_ge(PEc, 1)                       # 13 alpha
            vector.scalar_tensor_tensor(
                out=scr[:], in0=s[:], scalar=-2.0, in1=ls2[:],
                op0=ALU.mult, op1=ALU.add, accum_out=ST2[:, 6:7],
            ).then_inc(DVEc, 1)._wait_ge(DVEc, 12)                     # 14 -2C1
            vector.scalar_tensor_tensor(
                out=b1[:], in0=ls2[:], scalar=0.5, in1=lt[:],
                op0=ALU.mult, op1=ALU.subtract, accum_out=ST2[:, 3:4],
            ).then_inc(DVEc, 1)._wait_ge(DVEc, 13)                     # 15 -C3
            vector.scalar_tensor_tensor(
                out=d1[:], in0=scr[:], scalar=-0.5, in1=s[:],
                op0=ALU.mult, op1=ALU.mult, accum_out=ST2[:, 1:2],
            ).then_inc(DVEc, 1)._wait_ge(DVEc, 14)                     # 16 C2
            vector.scalar_tensor_tensor(
                out=d2[:], in0=b1[:], scalar=1.0, in1=b1[:],
                op0=ALU.mult, op1=ALU.mult, accum_out=ST2[:, 5:6],
            ).then_inc(DVEc, 1)._wait_ge(DVEc, 15)                     # 17 C4
            vector.wait_ge(DVEc, 16)
            vector.scalar_tensor_tensor(
                out=d3[:], in0=lt[:], scalar=1.0, in1=lnw[:],
                op0=ALU.mult, op1=ALU.mult, accum_out=ST2[:, 8:9],
            ).then_inc(DVEc, 1)._wait_ge(ACTc, 3)                      # 18 q2
            # r0-only side chain: t1 = 2B-2r0, den = r0*t1, invden = 1/den
            vector.tensor_scalar(
                out=r0s[0:1, 0:1], in0=psR0[:], scalar1=-2.0, scalar2=2.0 * Bf,
                op0=ALU.mult, op1=ALU.add,
            ).then_inc(DVEc, 1)._wait_ge(DVEc, 17)                     # 19 t1
            vector.tensor_scalar(
                out=qcoef[0:1, 2:3], in0=psR0[:], scalar1=-2.0 / (Bf * Bf),
                scalar2=1.0 / Bf, op0=ALU.mult, op1=ALU.add,
            ).then_inc(DVEc, 1)._wait_ge(DVEc, 17)                     # 20 beta
            vector.tensor_scalar(
                out=qcoef[0:1, 0:1], in0=psR0[:], scalar1=-0.5 / (Bf * Bf),
                scalar2=None, op0=ALU.mult,
            ).then_inc(DVEc, 1)._wait_ge(DVEc, 17)                     # 21 qc0
            vector.scalar_tensor_tensor(
                out=r0s[0:1, 1:2], in0=psR0[:], scalar=1.0, in1=r0s[0:1, 0:1],
                op0=ALU.mult, op1=ALU.mult,
            ).then_inc(DVEc, 1)._wait_ge(DVEc, 20)                     # 22 den
            vector.wait_ge(DVEc, 21)
            vector.scalar_tensor_tensor(
                out=prodA[:],
                in0=rc6[:].rearrange("p (a b) -> p a b", b=2)[:, :, 0:1],
                scalar=1.0,
                in1=rc6[:].rearrange("p (a b) -> p a b", b=2)[:, :, 1:2],
                op0=ALU.mult, op1=ALU.mult, accum_out=Ft[0:1, 0:1],
            ).then_inc(DVEc, 1)._wait_ge(ACTc, 4)                      # 23 num
            vector.reciprocal(
                r0s[0:1, 2:3], r0s[0:1, 1:2]
            ).then_inc(DVEc, 1)._wait_ge(DVEc, 22)                     # 24 invden
            vector.wait_ge(DVEc, 23)
            vector.scalar_tensor_tensor(
                out=prodB[:], in0=psQ[:], scalar=1.0, in1=qcoef[:],
                op0=ALU.mult, op1=ALU.mult, accum_out=QG[0:1, 0:1],
            ).then_inc(DVEc, 1)._wait_ge(PEc, 3)                       # 25 qdot
            vector.scalar_tensor_tensor(
                out=Gt[0:1, 0:2],
                in0=Ft[0:1, 0:1].broadcast_to([1, 2]),
                scalar=r0s[0:1, 2:3],
                in1=QG[0:1, 0:2],
                op0=ALU.mult, op1=ALU.add,
            ).then_inc(DVEc, 1)._wait_ge(DVEc, 25)                     # 26 G=[cls,pen]

        @block.tensor
        def _(tensor):
            # r0-only reduce, early: feeds invden/alpha/beta side chain
            tensor.matmul(
                psR0[:], ST2[:, 0:1], onescol[:]
            ).then_inc(PEc, 1)._wait_ge(DVEc, 9)
            tensor.wait_ge(Pd, 16)
            tensor.matmul(
                psR[:], onescol[:], ST2[:, 0:6]
            ).then_inc(PEc, 1)._wait_ge(DVEc, 17)
            tensor.wait_ge(ACTc, 3)
            tensor.matmul(
                psQ[:], onescol[:], ST2[:, 6:9]
            ).then_inc(PEc, 1)._wait_ge(DVEc, 18)

    import os
    nc.compile()
    _surgery(nc, mybir, strip_barriers=os.environ.get("STRIP_BARRIERS", "1") == "1")
    return nc


def _surgery(nc, mybir, strip_barriers=True):
    import json as _json

    # (a) Retarget the auto-inserted act table load to set 6
    # (natural_log_exp_and_others) which holds BOTH Exp and Ln; drop any
    # further loads (they carry no semaphore waits/updates).
    _COMBINED_EXP_LN_SET = 6
    for blk in nc.main_func.blocks:
        loads = [
            i for i in blk.instructions
            if isinstance(i, mybir.InstLoadActFuncSet)
        ]
        if not loads:
            continue
        assert all(not i.has_wait() and not i.has_update() for i in loads)
        loads[0].act_func_set_id = _COMBINED_EXP_LN_SET
        drop = {id(i) for i in loads[1:]}
        kept = [i for i in blk.instructions if id(i) not in drop]
        del blk.instructions[:]
        blk.instructions.extend(kept)

    # (b) Drop Bass.__init__'s unconditional const-AP memsets: nothing in
    # this kernel reads them (biases come from zerocol/onescol).
    for blk in nc.main_func.blocks:
        kept = []
        for i in blk.instructions:
            if isinstance(i, mybir.InstMemset) and not i.has_wait() and not i.has_update():
                j = _json.loads(mybir.instruction_to_pretty_json_string(i))
                memref = j.get("outs", [{}])[0].get("memref", "")
                if isinstance(memref, str) and memref.startswith("const-"):
                    continue
            kept.append(i)
        if len(kept) != len(blk.instructions):
            del blk.instructions[:]
            blk.instructions.extend(kept)

    # (c) Strip the entry/exit gather-release barrier protocol: this is a
    # single-shot kernel whose cross-engine ordering is fully expressed by
    # its own counter semaphores, so engines may start immediately and halt
    # after their last real instruction.
    if not strip_barriers:
        return

    def _is_barrier_sync(entry):
        return getattr(entry, "ant_name", "").startswith("barrier_")

    for blk in nc.main_func.blocks:
        kept = []
        for i in blk.instructions:
            si = i.sync_info
            waits = list(si.on_wait) if si else []
            upds = list(si.on_update) if si else []
            bw = [w for w in waits if _is_barrier_sync(w)]
            bu = [u for u in upds if _is_barrier_sync(u)]
            if not bw and not bu:
                kept.append(i)
                continue
            if isinstance(i, mybir.InstEventSemaphore):
                # pure barrier sync op: only keep if it carries non-barrier
                # syncs too (it never does in the generated prologue/epilogue)
                if len(bw) == len(waits) and len(bu) == len(upds):
                    continue
            # keep the instruction (e.g. Drain) minus its barrier syncs
            del si.on_wait[:]
            si.on_wait.extend([w for w in waits if not _is_barrier_sync(w)])
            del si.on_update[:]
            si.on_update.extend([u for u in upds if not _is_barrier_sync(u)])
            kept.append(i)
        del blk.instructions[:]
        blk.instructions.extend(kept)


def _in_map(output: np.ndarray, labels: np.ndarray) -> dict:
    return {
        "output": np.ascontiguousarray(output, dtype=np.float32),
        "labels": np.ascontiguousarray(labels, dtype=np.float32),
    }


def kernel(output: np.ndarray, labels: np.ndarray) -> np.ndarray:
    global _nc_cache
    from concourse.bass_utils import run_bass_kernel_spmd

    if _nc_cache is None:
        _nc_cache = build_nc()
    res = run_bass_kernel_spmd(_nc_cache, [_in_map(output, labels)], core_ids=[0])
    g = res.results[0]["out"]
    return np.asarray(g, dtype=np.float32).reshape(64)[0:2].copy()


# revision 13
# speedup vs baseline: 1.2984x; 1.2074x over previous
"""Trainium2 Bass kernel for nn_CWAUCHLoss (pairwise AUC hinge + class-weighted CE).

Math: with s = sigmoid(output[:, 0]), lab = labels[:, 0], LAMB == 2, the
O(B^2) pairwise penalty collapses to 3 pairwise products of masked sums,
and the CE part is linear in the two log-sums given the label count:

  num  = r0*C2 + 2*C1*C3 + (B-r0)*C4      pen = num / (2*r0*(B-r0))
  fpcls = alpha*q1 + beta*q2 + alpha*C1   cls = fpcls + pen
  where C1 = sum s*(1-lab)       C2 = sum s^2*(1-lab)
        C3 = sum lab*(1-s)       C4 = sum lab*(1-s)^2      r0 = sum lab
        q1 = sum ln(1+e^-s)      q2 = sum lab*ln(1+e^-s)
        alpha = r0/B^2           beta = (B-2*r0)/B^2

Device schedule (one NeuronCore, batch as 128 partitions x 64 lanes):
  - per-partition combo-columns accumulate in ST2[128, 9] laid out as
    [r0, C2, -2C1, -C3, 64-r0_p, C4 | -2C1, q1, q2] so that a single
    stride-2-paired STT dot gives num, and a single 3-term STT dot against
    [-a/2, a, b] gives fpcls (sign folds make every op expressible as one
    scalar_tensor_tensor).
  - cross-partition reduction is two tiny matmuls against a ones column
    (cols 0:6 -> psR as soon as C4 lands, cols 6:9 -> psQ after q2); an
    r0-only matmul runs early so 1/den, alpha, beta are ready off-path.
  - the output travels through a PREPARED SWDGE scatter-add descriptor:
    desc-gen (~1us Pool time) happens during the input DMA; after G is
    written, trigger_dma fires the 256B payload immediately - skipping
    the 625ns HWDGE desc-gen + 650ns DGE delay a fresh dma_start pays.
    o_d is zero-initialized by an early HWDGE DMA so add == store.
  - ALL transcendentals (Exp, Ln) use act table set 6
    (natural_log_exp_and_others); post-compile surgery retargets the
    auto-inserted table load (same trick as before).
  - post-compile surgery also strips Bass's entry/exit barrier protocol
    (gather/release EventSemaphores + Drain sync): single-shot kernel,
    engines start immediately and halt after their last instruction.
"""

import numpy as np

B = 8192
P = 128
N = B // P  # 64 elements per partition

_nc_cache = None


def build_nc():
    from contextlib import ExitStack

    import concourse.bacc as bacc
    import concourse.mybir as mybir

    f32 = mybir.dt.float32
    i32 = mybir.dt.int32
    AF = mybir.ActivationFunctionType
    ALU = mybir.AluOpType
    AX = mybir.AxisListType

    nc = bacc.Bacc(None, target_bir_lowering=False, debug=False)
    x_d = nc.dram_tensor("output", [B, 2], f32, kind="ExternalInput")
    l_d = nc.dram_tensor("labels", [B, 1], f32, kind="ExternalInput")
    o_d = nc.dram_tensor("out", [2, 128], f32, kind="ExternalOutput")

    Bf = float(B)

    with ExitStack() as ctx:
        e = ctx.enter_context
        xt = e(nc.sbuf_tensor([P, N, 2], f32))
        lt = e(nc.sbuf_tensor([P, N], f32))
        e1 = e(nc.sbuf_tensor([P, N], f32))
        p1 = e(nc.sbuf_tensor([P, N], f32))
        s = e(nc.sbuf_tensor([P, N], f32))
        e2 = e(nc.sbuf_tensor([P, N], f32))
        lnw = e(nc.sbuf_tensor([P, N], f32))
        ls2 = e(nc.sbuf_tensor([P, N], f32))
        scr = e(nc.sbuf_tensor([P, N], f32))
        b1 = e(nc.sbuf_tensor([P, N], f32))
        d1 = e(nc.sbuf_tensor([P, N], f32))
        d2 = e(nc.sbuf_tensor([P, N], f32))
        d3 = e(nc.sbuf_tensor([P, N], f32))
        ST2 = e(nc.sbuf_tensor([P, 9], f32))
        onescol = e(nc.sbuf_tensor([P, 1], f32))
        zerocol = e(nc.sbuf_tensor([P, 1], f32))
        Gt = e(nc.sbuf_tensor([P, 2], f32))
        cidx = e(nc.sbuf_tensor([P, 2], i32))
        qcoef = e(nc.sbuf_tensor([1, 3], f32))
        r0s = e(nc.sbuf_tensor([1, 4], f32))  # [t1, den, invden, spare]
        Ft = e(nc.sbuf_tensor([1, 1], f32))
        QG = e(nc.sbuf_tensor([1, 2], f32))
        rc6 = e(nc.sbuf_tensor([1, 6], f32))
        prodA = e(nc.sbuf_tensor([1, 3, 1], f32))
        prodB = e(nc.sbuf_tensor([1, 3], f32))
        psR0 = e(nc.psum_tensor([1, 1], f32))
        psR = e(nc.psum_tensor([1, 6], f32))
        psQ = e(nc.psum_tensor([1, 3], f32))
        d_x = e(nc.semaphore("d_x"))
        d_l = e(nc.semaphore("d_l"))
        d_o = e(nc.semaphore("d_o"))
        Pp = e(nc.semaphore("Pp"))
        Pd = e(nc.semaphore("Pd"))
        ACTc = e(nc.semaphore("ACTc"))
        DVEc = e(nc.semaphore("DVEc"))
        PEc = e(nc.semaphore("PEc"))
        block = e(nc.Block())

        import os
        out_mode = os.environ.get("OUT_MODE", "trig")

        @block.sync
        def _(sync):
            # x first: it gates the whole compute chain.
            sync.dma_start(
                xt[:], x_d.ap().rearrange("(p n) c -> p n c", p=P)
            ).then_inc(d_x, 16)
            sync.dma_start(
                lt[:], l_d.ap().rearrange("(p n) c -> p (n c)", p=P)
            ).then_inc(d_l, 16)
            if out_mode == "dma":
                sync.wait_ge(DVEc, 24)
                sync.dma_start(
                    o_d.ap().rearrange("b o -> (b o)").rearrange("(a x) -> a x", a=1),
                    Gt[0:1, 0:2], max_dma_last_dim=None,
                ).then_inc(d_o, 16)
                sync.wait_ge(d_o, 16)

        @block.gpsimd
        def _(gpsimd):
            if out_mode == "trig":
                # Pre-generate the output descriptors during the input DMA
                # window; trigger_dma later fires them with no desc-gen or
                # DGE-init delay on the tail. kv_writeback with batch=2,
                # d_head=128, n_ctx=1 writes out[b, p] = Gt[p, b]: row 0
                # col 0 = cls, row 1 col 0 = pen (cols 1.. get Gt's zeros).
                gpsimd.wait_ge(DVEc, 1)
                gpsimd.kv_writeback(
                    out_ap=o_d.ap().rearrange("b (i o c) -> b i o c", i=1, c=1),
                    in_ap=Gt[:].rearrange("p (i b c) -> p i b c", i=1, c=1),
                    ctx_idxs_ap=cidx[:],
                    prepare_only=True,
                    sem=d_o,
                ).then_inc(Pp, 1)
            # duplicate the -2C1 accum column into the num-pair slot (Pool is
            # idle here; keeps the DVE column pipeline one op shorter)
            gpsimd.tensor_scalar(
                out=ST2[:, 2:3], in0=ST2[:, 6:7], scalar1=1.0, scalar2=None,
                op0=ALU.mult,
            ).then_inc(Pd, 16)._wait_ge(DVEc, 12)
            if out_mode == "trig":
                gpsimd.wait_ge(Pp, 1)
                gpsimd.trigger_dma(count=1)._wait_ge(DVEc, 24)
                gpsimd.wait_ge(d_o, 16)

        @block.scalar
        def _(scalar):
            scalar.wait_ge(DVEc, 3)  # zerocol/onescol biases
            scalar.activation(
                e1[:], xt[:, :, 0], AF.Exp, scale=-1.0, bias=zerocol[:, 0:1]
            ).then_inc(ACTc, 1)._wait_ge(d_x, 16)  # 1
            scalar.activation(
                e2[:], s[:], AF.Exp, scale=-1.0, bias=zerocol[:, 0:1]
            ).then_inc(ACTc, 1)._wait_ge(DVEc, 8)  # 2
            scalar.activation(
                lnw[:], e2[:], AF.Ln, bias=onescol[:, 0:1],
                accum_out=ST2[:, 7:8],
            ).then_inc(ACTc, 1)._wait_ge(ACTc, 2)  # 3 (accum -> q1)
            # stage psR to SBUF for the num dot (DVE may read only one PSUM
            # operand per op; ACT is idle here and reads PSUM fast)
            scalar.activation(
                rc6[:], psR[:], AF.Copy, scale=1.0, bias=0.0
            ).then_inc(ACTc, 1)._wait_ge(PEc, 2)  # 4

        @block.vector
        def _(vector):
            # DVEc is a monotone completion counter: a wait >= k orders this
            # op after ALL DVE ops <= k. Waits target op k-2 or earlier
            # wherever possible (free in the pipeline); only true
            # immediately-adjacent RAW pairs pay the sem round-trip.
            vector.memset(cidx[:], 0).then_inc(DVEc, 1)                          # 1
            vector.memset(onescol[:], 1.0).then_inc(DVEc, 1)._wait_ge(DVEc, 1)   # 2
            vector.memset(zerocol[:], 0.0).then_inc(DVEc, 1)._wait_ge(DVEc, 2)   # 3
            vector.memset(QG[:], 0.0).then_inc(DVEc, 1)._wait_ge(DVEc, 3)        # 4
            vector.memset(Gt[:], 0.0).then_inc(DVEc, 1)._wait_ge(DVEc, 4)        # 5
            # s = sigmoid(x0) = 1/(1+e1)
            vector.wait_ge(DVEc, 5)
            vector.tensor_scalar_add(
                p1[:], e1[:], 1.0
            ).then_inc(DVEc, 1)._wait_ge(ACTc, 1)                      # 6
            vector.tensor_reduce(
                ST2[:, 0:1], lt[:], axis=AX.X, op=ALU.add
            ).then_inc(DVEc, 1)._wait_ge(d_l, 16)                      # 7 r0
            vector.reciprocal(s[:], p1[:]).then_inc(DVEc, 1)._wait_ge(DVEc, 6)  # 8
            vector.tensor_scalar(
                out=ST2[:, 4:5], in0=ST2[:, 0:1], scalar1=-1.0, scalar2=float(N),
                op0=ALU.mult, op1=ALU.add,
            ).then_inc(DVEc, 1)._wait_ge(DVEc, 7)                      # 9 Bmr col
            vector.scalar_tensor_tensor(
                out=ls2[:], in0=lt[:], scalar=2.0, in1=s[:],
                op0=ALU.mult, op1=ALU.mult,
            ).then_inc(DVEc, 1)._wait_ge(DVEc, 8)                      # 10 ls2=2*lab*s
            vector.tensor_scalar(
                out=qcoef[0:1, 1:2], in0=psR0[:], scalar1=1.0 / (Bf * Bf),
                scalar2=None, op0=ALU.mult,
            ).then_inc(DVEc, 1)._wait_ge(PEc, 1)                       # 11 alpha
            vector.scalar_tensor_tensor(
                out=scr[:], in0=s[:], scalar=-2.0, in1=ls2[:],
                op0=ALU.mult, op1=ALU.add, accum_out=ST2[:, 6:7],
            ).then_inc(DVEc, 1)._wait_ge(DVEc, 10)                     # 12 -2C1
            vector.scalar_tensor_tensor(
                out=b1[:], in0=ls2[:], scalar=0.5, in1=lt[:],
                op0=ALU.mult, op1=ALU.subtract, accum_out=ST2[:, 3:4],
            ).then_inc(DVEc, 1)._wait_ge(DVEc, 11)                     # 13 -C3
            vector.scalar_tensor_tensor(
                out=d1[:], in0=scr[:], scalar=-0.5, in1=s[:],
                op0=ALU.mult, op1=ALU.mult, accum_out=ST2[:, 1:2],
            ).then_inc(DVEc, 1)._wait_ge(DVEc, 12)                     # 14 C2
            vector.scalar_tensor_tensor(
                out=d2[:], in0=b1[:], scalar=1.0, in1=b1[:],
                op0=ALU.mult, op1=ALU.mult, accum_out=ST2[:, 5:6],
            ).then_inc(DVEc, 1)._wait_ge(DVEc, 13)                     # 15 C4
            vector.wait_ge(DVEc, 14)
            vector.scalar_tensor_tensor(
                out=d3[:], in0=lt[:], scalar=1.0, in1=lnw[:],
                op0=ALU.mult, op1=ALU.mult, accum_out=ST2[:, 8:9],
            ).then_inc(DVEc, 1)._wait_ge(ACTc, 3)                      # 16 q2
            # r0-only side chain: t1 = 2B-2r0, den = r0*t1, invden = 1/den
            vector.tensor_scalar(
                out=r0s[0:1, 0:1], in0=psR0[:], scalar1=-2.0, scalar2=2.0 * Bf,
                op0=ALU.mult, op1=ALU.add,
            ).then_inc(DVEc, 1)._wait_ge(DVEc, 15)                     # 17 t1
            vector.tensor_scalar(
                out=qcoef[0:1, 2:3], in0=psR0[:], scalar1=-2.0 / (Bf * Bf),
                scalar2=1.0 / Bf, op0=ALU.mult, op1=ALU.add,
            ).then_inc(DVEc, 1)._wait_ge(DVEc, 15)                     # 18 beta
            vector.tensor_scalar(
                out=qcoef[0:1, 0:1], in0=psR0[:], scalar1=-0.5 / (Bf * Bf),
                scalar2=None, op0=ALU.mult,
            ).then_inc(DVEc, 1)._wait_ge(DVEc, 15)                     # 19 qc0
            vector.scalar_tensor_tensor(
                out=r0s[0:1, 1:2], in0=psR0[:], scalar=1.0, in1=r0s[0:1, 0:1],
                op0=ALU.mult, op1=ALU.mult,
            ).then_inc(DVEc, 1)._wait_ge(DVEc, 18)                     # 20 den
            vector.wait_ge(DVEc, 19)
            vector.scalar_tensor_tensor(
                out=prodA[:],
                in0=rc6[:].rearrange("p (a b) -> p a b", b=2)[:, :, 0:1],
                scalar=1.0,
                in1=rc6[:].rearrange("p (a b) -> p a b", b=2)[:, :, 1:2],
                op0=ALU.mult, op1=ALU.mult, accum_out=Ft[0:1, 0:1],
            ).then_inc(DVEc, 1)._wait_ge(ACTc, 4)                      # 21 num
            vector.reciprocal(
                r0s[0:1, 2:3], r0s[0:1, 1:2]
            ).then_inc(DVEc, 1)._wait_ge(DVEc, 20)                     # 22 invden
            vector.wait_ge(DVEc, 21)
            vector.scalar_tensor_tensor(
                out=prodB[:], in0=psQ[:], scalar=1.0, in1=qcoef[:],
                op0=ALU.mult, op1=ALU.mult, accum_out=QG[0:1, 0:1],
            ).then_inc(DVEc, 1)._wait_ge(PEc, 3)                       # 23 qdot
            vector.scalar_tensor_tensor(
                out=Gt[0:1, 0:2],
                in0=Ft[0:1, 0:1].broadcast_to([1, 2]),
                scalar=r0s[0:1, 2:3],
                in1=QG[0:1, 0:2],
                op0=ALU.mult, op1=ALU.add,
            ).then_inc(DVEc, 1)._wait_ge(DVEc, 23)                     # 24 G=[cls,pen]

        @block.tensor
        def _(tensor):
            # r0-only reduce, early: feeds invden/alpha/beta side chain
            tensor.matmul(
                psR0[:], ST2[:, 0:1], onescol[:]
            ).then_inc(PEc, 1)._wait_ge(DVEc, 7)
            tensor.wait_ge(Pd, 16)
            tensor.matmul(
                psR[:], onescol[:], ST2[:, 0:6]
            ).then_inc(PEc, 1)._wait_ge(DVEc, 15)
            tensor.wait_ge(ACTc, 3)
            tensor.matmul(
                psQ[:], onescol[:], ST2[:, 6:9]
            ).then_inc(PEc, 1)._wait_ge(DVEc, 16)

    import os
    nc.compile()
    _surgery(nc, mybir, strip_barriers=os.environ.get("STRIP_BARRIERS", "1") == "1")
    return nc


def _surgery(nc, mybir, strip_barriers=True):
    import json as _json

    # (a) Retarget the auto-inserted act table load to set 6
    # (natural_log_exp_and_others) which holds BOTH Exp and Ln; drop any
    # further loads (they carry no semaphore waits/updates).
    _COMBINED_EXP_LN_SET = 6
    for blk in nc.main_func.blocks:
        loads = [
            i for i in blk.instructions
            if isinstance(i, mybir.InstLoadActFuncSet)
        ]
        if not loads:
            continue
        assert all(not i.has_wait() and not i.has_update() for i in loads)
        loads[0].act_func_set_id = _COMBINED_EXP_LN_SET
        drop = {id(i) for i in loads[1:]}
        kept = [i for i in blk.instructions if id(i) not in drop]
        del blk.instructions[:]
        blk.instructions.extend(kept)

    # (b) Drop Bass.__init__'s unconditional const-AP memsets: nothing in
    # this kernel reads them (biases come from zerocol/onescol).
    for blk in nc.main_func.blocks:
        kept = []
        for i in blk.instructions:
            if isinstance(i, mybir.InstMemset) and not i.has_wait() and not i.has_update():
                j = _json.loads(mybir.instruction_to_pretty_json_string(i))
                memref = j.get("outs", [{}])[0].get("memref", "")
                if isinstance(memref, str) and memref.startswith("const-"):
                    continue
            kept.append(i)
        if len(kept) != len(blk.instructions):
            del blk.instructions[:]
            blk.instructions.extend(kept)

    # (c) Strip the entry/exit gather-release barrier protocol: this is a
    # single-shot kernel whose cross-engine ordering is fully expressed by
    # its own counter semaphores, so engines may start immediately and halt
    # after their last real instruction.
    if not strip_barriers:
        return

    def _is_barrier_sync(entry):
        return getattr(entry, "ant_name", "").startswith("barrier_")

    for blk in nc.main_func.blocks:
        kept = []
        for i in blk.instructions:
            si = i.sync_info
            waits = list(si.on_wait) if si else []
            upds = list(si.on_update) if si else []
            bw = [w for w in waits if _is_barrier_sync(w)]
            bu = [u for u in upds if _is_barrier_sync(u)]
            if not bw and not bu:
                kept.append(i)
                continue
            if isinstance(i, mybir.InstEventSemaphore):
                # pure barrier sync op: only keep if it carries non-barrier
                # syncs too (it never does in the generated prologue/epilogue)
                if len(bw) == len(waits) and len(bu) == len(upds):
                    continue
            # keep the instruction (e.g. Drain) minus its barrier syncs
            del si.on_wait[:]
            si.on_wait.extend([w for w in waits if not _is_barrier_sync(w)])
            del si.on_update[:]
            si.on_update.extend([u for u in upds if not _is_barrier_sync(u)])
            kept.append(i)
        del blk.instructions[:]
        blk.instructions.extend(kept)


def _in_map(output: np.ndarray, labels: np.ndarray) -> dict:
    return {
        "output": np.ascontiguousarray(output, dtype=np.float32),
        "labels": np.ascontiguousarray(labels, dtype=np.float32),
    }


def kernel(output: np.ndarray, labels: np.ndarray) -> np.ndarray:
    global _nc_cache
    from concourse.bass_utils import run_bass_kernel_spmd

    if _nc_cache is None:
        _nc_cache = build_nc()
    res = run_bass_kernel_spmd(_nc_cache, [_in_map(output, labels)], core_ids=[0])
    g = res.results[0]["out"]
    return np.asarray(g, dtype=np.float32).reshape(2, 128)[:, 0].copy()
